# revision 18
# baseline (speedup 1.0000x reference)
"""Trainium2 Bass kernel for CudaTensorProduct (e3nn-style COO tensor product).

Computation: out[b, o] = sum_k cb[k] * in1[b, idx1[k]] * in2[b, idx2[k]]
  in1/in2: (16384, 32) f32, out: (16384, 1024) f32, nnz=4528.

Strategy (per core, pure data-parallel over batch, 2048 rows/core):
  The l-structure (ls1=ls2=[0,1,2,3]x2) factorizes: permute in1 columns into
  4 "i-sets" of 8 ({l1=0,3} and {l1=1,2} per copy); in2 columns split into
  2 "j-sets" of 16 (the two copies). Every (l1,l2,l3) coupling then lives in
  exactly one of the 8 pair-tiles q=(a,b) = iset_a x jset_b, each with
  exactly 128 (i,j) pairs AND exactly 128 output rows -> the coefficient
  matrix W is block-diagonal over q with 128x128 blocks.

  Per core (inputs host-pre-transposed to in12T (64, 2048) bf16):
    R1_a = E1a.T @ in12T   (PE; 8 i-rows each replicated 16x) -- shared by
                            both j-sets; 4 matmuls per 512-chunk total
    R2_b = E2b.T @ in12T   (PE; 16 j-rows tiled 8x) -- shared by 4 i-sets
    cast R PSUM->SBUF bf16 (ACT/GPSIMD)
    U_q  = R1_a * R2_b     (DVE scalar_tensor_tensor, all-bf16 SBUF = 4x mode)
    outT_q = W_q.T @ U_q   (PE, bf16, PSUM fp32 accum)
    cast out PSUM->SBUF bf16 (ACT/GPSIMD/DVE), DMA bf16 to HBM.

  14 weight loads + 56 matmuls of 512 cols per core; host un-permutes and
  upcasts the bf16 output to fp32 during the unshard (pure layout).
"""

import os
import sys
import numpy as np
import ml_dtypes

sys.path.insert(0, "/opt/trn_rl_repo")

import concourse.bass as bass
import concourse.mybir as mybir
import concourse.tile as tile
from concourse import bacc
from concourse.bass_utils import run_bass_kernel_spmd

N_CORES = 8
B = 16384
BC = B // N_CORES          # 2048 batch rows per core
D1 = 32
D2 = 32
DOUT = D1 * D2             # 1024
NQ = 8                     # pair-tiles (4 i-sets x 2 j-sets)
CHUNK = 512                # batch columns per matmul
NCHUNK = BC // CHUNK       # 4
F32 = mybir.dt.float32
BF16 = mybir.dt.bfloat16
MULT = mybir.AluOpType.mult

LS = [0, 1, 2, 3, 0, 1, 2, 3]


# ----------------------------------------------------------------------------
# Host-side table preprocessing
# ----------------------------------------------------------------------------

def _build_tables(idx1, idx2, out_idx, cb_vals):
    """Build the factorized layout.

    Returns (iperm, e12, w, rows_map):
      iperm: (32,) permutation of in1 columns (iset-major).
      e12: (64, 6*128) bf16 -- E1a at cols a*128.. (a=0..3), E2b at
           cols (4+b)*128.. (b=0,1); rows index in12T partitions.
      w:   (128, 8*128) bf16 -- w[p, q*128+m] = coefficient for pair p
           (p = i_local*16 + j_local) into scratch out row q*128+m.
      rows_map: (1024,) int -- scratch row -> real out column.
    """
    idx1 = np.asarray(idx1, np.int64)
    idx2 = np.asarray(idx2, np.int64)
    out_idx = np.asarray(out_idx, np.int64)
    cb = np.asarray(cb_vals, np.float64)

    offs, blocks = 0, []
    for l in LS:
        blocks.append(list(range(offs, offs + 2 * l + 1)))
        offs += 2 * l + 1
    isets = [blocks[0] + blocks[3], blocks[1] + blocks[2],
             blocks[4] + blocks[7], blocks[5] + blocks[6]]
    jsets = [list(range(16)), list(range(16, 32))]
    imap = {c: (a, il) for a, s in enumerate(isets) for il, c in enumerate(s)}
    jmap = {c: (b, jl) for b, s in enumerate(jsets) for jl, c in enumerate(s)}

    out_q = {}
    for k in range(len(cb)):
        a, _ = imap[int(idx1[k])]
        b, _ = jmap[int(idx2[k])]
        q = a * 2 + b
        o = int(out_idx[k])
        assert out_q.setdefault(o, q) == q, "coupling crosses pair-tiles"
    rows_map = np.zeros(NQ * 128, np.int64)
    out_local = {}
    for q in range(NQ):
        outs = sorted(o for o, qq in out_q.items() if qq == q)
        assert len(outs) == 128, (q, len(outs))
        for m, o in enumerate(outs):
            out_local[o] = m
            rows_map[q * 128 + m] = o

    e12 = np.zeros((64, 6 * 128), np.float32)
    for a in range(4):
        for p in range(128):
            e12[a * 8 + p // 16, a * 128 + p] = 1.0
    for b in range(2):
        for p in range(128):
            e12[32 + b * 16 + p % 16, (4 + b) * 128 + p] = 1.0

    w = np.zeros((128, NQ * 128), np.float64)
    for k in range(len(cb)):
        a, il = imap[int(idx1[k])]
        b, jl = jmap[int(idx2[k])]
        q = a * 2 + b
        p = il * 16 + jl
        m = out_local[int(out_idx[k])]
        w[p, q * 128 + m] += cb[k]

    iperm = np.concatenate([np.asarray(s) for s in isets])
    bf = ml_dtypes.bfloat16
    return iperm, e12.astype(bf), w.astype(np.float32).astype(bf), rows_map


# ----------------------------------------------------------------------------
# Device program
# ----------------------------------------------------------------------------

def _build_bass():
    nc = bacc.Bacc("TRN2", target_bir_lowering=False)

    in12h = nc.dram_tensor("in12h", [64, BC], BF16, kind="ExternalInput")
    e12 = nc.dram_tensor("e12", [64, 6 * 128], BF16, kind="ExternalInput")
    wgt = nc.dram_tensor("wgt", [128, NQ * 128], BF16, kind="ExternalInput")
    outT = nc.dram_tensor("outT", [NQ * 128, BC], BF16, kind="ExternalOutput")

    # R slots 0-3 = R1 (isets), 4-5 = R2 (jsets).
    # Emission program: 16 R-matmuls up front (PE p-state ramp), then mains
    # interleaved with the last two R slots so the PE never drains. GPSIMD
    # (slow, SBUF-only) gets the two muls consumed last, emitted early.
    # 'R' = slot matmuls+casts, 'M' = DVE mul, 'MG' = GPSIMD mul,
    # 'Q' = main matmuls + out casts + DMA for pair-tile (a, b).
    program = [
        ('R', 0), ('R', 4), ('R', 1), ('MG', (1, 0)), ('R', 5), ('MG', (0, 1)),
        ('M', (0, 0)), ('M', (1, 1)),
        ('Q', (0, 0)), ('R', 2), ('M', (2, 0)), ('M', (2, 1)),
        ('Q', (1, 1)), ('R', 3), ('M', (3, 1)), ('M', (3, 0)),
        ('Q', (2, 0)), ('Q', (3, 1)), ('Q', (2, 1)), ('Q', (3, 0)),
        ('Q', (1, 0)), ('Q', (0, 1)),
    ]
    gps_muls = {(0, 1), (1, 0)}
    # cast engine per op: R slots mostly alternate ACT/DVE; slot 2 all ACT
    # (keeps DVE free for muls); out casts ~11 ACT / 5 DVE, tail parallel.
    r_cast_plan = {0: 'av', 4: 'av', 1: 'av', 5: 'av', 2: 'aa', 3: 'av'}
    o_cast_seq = 'aavaavaavaavaaav'

    with tile.TileContext(nc) as tc:
        with (
            tc.tile_pool(name="const", bufs=1) as const_pool,
            tc.tile_pool(name="work", bufs=1) as work_pool,
            tc.tile_pool(name="ps_r", bufs=2, space="PSUM") as ps_r_pool,
            tc.tile_pool(name="ps_o", bufs=2, space="PSUM") as ps_o_pool,
        ):
            # issue the input DMAs from four different engine queues so they
            # start in parallel instead of serializing on the Sync sequencer
            x_sb = work_pool.tile([64, BC], BF16)
            nc.sync.dma_start(out=x_sb[:, : BC // 2], in_=in12h.ap()[:, : BC // 2])
            nc.gpsimd.dma_start(out=x_sb[:, BC // 2 :], in_=in12h.ap()[:, BC // 2 :])
            e_sb = const_pool.tile([64, 6 * 128], BF16)
            nc.scalar.dma_start(out=e_sb[:], in_=e12.ap())
            w_sb = const_pool.tile([128, NQ * 128], BF16)
            nc.sync.dma_start(out=w_sb[:], in_=wgt.ap())

            # PE p-state warmup: garbage matmuls on a zeroed tile while the
            # input DMAs are in flight -- the DVFS governor needs sustained
            # utilization before it clocks the PE up, so start spinning early.
            wu_sb = work_pool.tile([64, 640], BF16)
            nc.gpsimd.memset(wu_sb[:], 0)
            for wu in range(8):
                ps = ps_r_pool.tile([128, 2 * CHUNK], F32)
                for ci in range(2):
                    nc.tensor.matmul(
                        ps[:, ci * CHUNK : (ci + 1) * CHUNK],
                        lhsT=wu_sb[:, : 128],
                        rhs=wu_sb[:, 128 : 128 + CHUNK],
                        start=True,
                        stop=True,
                    )

            r_sb = work_pool.tile([128, 6 * BC], BF16)
            u_sb = work_pool.tile([128, NQ * BC], BF16)
            osb = work_pool.tile([128, NQ * BC], BF16)

            def emit_cast(code, dst, ps):
                if code == 'a':
                    nc.scalar.copy(out=dst, in_=ps[:])
                else:
                    nc.vector.tensor_copy(dst, ps[:])

            oci = [0]
            for kind, arg in program:
                if kind == 'R':
                    slot = arg
                    for h in range(2):  # halves of BC: 2 chunks each
                        ps = ps_r_pool.tile([128, 2 * CHUNK], F32)
                        for ci in range(2):
                            c = h * 2 + ci
                            nc.tensor.matmul(
                                ps[:, ci * CHUNK : (ci + 1) * CHUNK],
                                lhsT=e_sb[:, slot * 128 : (slot + 1) * 128],
                                rhs=x_sb[:, c * CHUNK : (c + 1) * CHUNK],
                                start=True,
                                stop=True,
                            )
                        emit_cast(
                            r_cast_plan[slot][h],
                            r_sb[:, slot * BC + h * 1024 : slot * BC + (h + 1) * 1024],
                            ps,
                        )
                elif kind in ('M', 'MG'):
                    a, b = arg
                    q = a * 2 + b
                    out_ap = u_sb[:, q * BC : (q + 1) * BC]
                    in0 = r_sb[:, a * BC : (a + 1) * BC]
                    in1 = r_sb[:, (4 + b) * BC : (5 + b) * BC]
                    if kind == 'MG':
                        # Pool engine: only plain TensorTensor is supported
                        nc.gpsimd.tensor_mul(out_ap, in0, in1)
                    else:
                        nc.vector.tensor_mul(out_ap, in0, in1)
                else:  # 'Q': mains + out casts + per-half DMA
                    a, b = arg
                    q = a * 2 + b
                    for h in range(2):
                        ps = ps_o_pool.tile([128, 2 * CHUNK], F32)
                        for ci in range(2):
                            c = h * 2 + ci
                            nc.tensor.matmul(
                                ps[:, ci * CHUNK : (ci + 1) * CHUNK],
                                lhsT=w_sb[:, q * 128 : (q + 1) * 128],
                                rhs=u_sb[:, q * BC + c * CHUNK : q * BC + (c + 1) * CHUNK],
                                start=True,
                                stop=True,
                            )
                        emit_cast(
                            o_cast_seq[oci[0] % len(o_cast_seq)],
                            osb[:, q * BC + h * 1024 : q * BC + (h + 1) * 1024],
                            ps,
                        )
                        oci[0] += 1
                        nc.sync.dma_start(
                            out=outT.ap()[
                                q * 128 : (q + 1) * 128, h * 1024 : (h + 1) * 1024
                            ],
                            in_=osb[:, q * BC + h * 1024 : q * BC + (h + 1) * 1024],
                        )
    nc.compile()
    return nc


# ----------------------------------------------------------------------------
# Entry point
# ----------------------------------------------------------------------------

_CACHE = {}


def kernel(in1, in2, cb_vals, idx1, idx2, out_idx):
    in1 = np.ascontiguousarray(np.asarray(in1, np.float32))
    in2 = np.ascontiguousarray(np.asarray(in2, np.float32))

    key = (
        np.asarray(idx1).tobytes(),
        np.asarray(idx2).tobytes(),
        np.asarray(out_idx).tobytes(),
        np.asarray(cb_vals).tobytes(),
    )
    kh = hash(key)
    if kh not in _CACHE:
        iperm, e12, w, rows_map = _build_tables(idx1, idx2, out_idx, cb_vals)
        nc = _build_bass()
        _CACHE[kh] = (nc, iperm, e12, w, rows_map)
    nc, iperm, e12, w, rows_map = _CACHE[kh]

    bf = ml_dtypes.bfloat16
    in1p = in1[:, iperm]
    in_maps = []
    for core in range(N_CORES):
        sl = slice(core * BC, (core + 1) * BC)
        in12h = np.ascontiguousarray(
            np.concatenate([in1p[sl], in2[sl]], axis=1).T.astype(bf)
        )  # (64, BC)
        in_maps.append({"in12h": in12h, "e12": e12, "wgt": w})

    trace = bool(int(os.environ.get("KERNEL_TRACE", "0")))
    res = run_bass_kernel_spmd(
        nc, in_maps, core_ids=list(range(N_CORES)), trace=trace
    )
    kernel.last_results = res

    out = np.empty((B, DOUT), np.float32)
    for core in range(N_CORES):
        shard = res.results[core]["outT"]  # (1024, BC) bf16 scratch layout
        out[core * BC : (core + 1) * BC][:, rows_map] = (
            np.asarray(shard).astype(np.float32).T
        )
    return out


# revision 19
# speedup vs baseline: 1.0974x; 1.0974x over previous
"""Trainium2 Bass kernel for CudaTensorProduct (e3nn-style COO tensor product).

Computation: out[b, o] = sum_k cb[k] * in1[b, idx1[k]] * in2[b, idx2[k]]
  in1/in2: (16384, 32) f32, out: (16384, 1024) f32, nnz=4528.

Strategy (per core, pure data-parallel over batch, 2048 rows/core):
  The l-structure (ls1=ls2=[0,1,2,3]x2) factorizes: permute in1 columns into
  4 "i-sets" of 8 ({l1=0,3} and {l1=1,2} per copy); in2 columns split into
  2 "j-sets" of 16 (the two copies). Every (l1,l2,l3) coupling then lives in
  exactly one of the 8 pair-tiles q=(a,b) = iset_a x jset_b, each with
  exactly 128 (i,j) pairs AND exactly 128 output rows -> the coefficient
  matrix W is block-diagonal over q with 128x128 blocks.

  Per core (inputs host-pre-transposed to in12T (64, 2048) bf16):
    R1_a = E1a.T @ in12T   (PE; 8 i-rows each replicated 16x) -- shared by
                            both j-sets; 4 matmuls per 512-chunk total
    R2_b = E2b.T @ in12T   (PE; 16 j-rows tiled 8x) -- shared by 4 i-sets
    cast R PSUM->SBUF bf16 (ACT/GPSIMD)
    U_q  = R1_a * R2_b     (DVE scalar_tensor_tensor, all-bf16 SBUF = 4x mode)
    outT_q = W_q.T @ U_q   (PE, bf16, PSUM fp32 accum)
    cast out PSUM->SBUF bf16 (ACT/GPSIMD/DVE), DMA bf16 to HBM.

  14 weight loads + 56 matmuls of 512 cols per core; host un-permutes and
  upcasts the bf16 output to fp32 during the unshard (pure layout).
"""

import os
import sys
import numpy as np
import ml_dtypes

sys.path.insert(0, "/opt/trn_rl_repo")

import concourse.bass as bass
import concourse.mybir as mybir
import concourse.tile as tile
from concourse import bacc
from concourse.bass_utils import run_bass_kernel_spmd

N_CORES = 8
B = 16384
BC = B // N_CORES          # 2048 batch rows per core
D1 = 32
D2 = 32
DOUT = D1 * D2             # 1024
NQ = 8                     # pair-tiles (4 i-sets x 2 j-sets)
CHUNK = 512                # batch columns per matmul
NCHUNK = BC // CHUNK       # 4
F32 = mybir.dt.float32
BF16 = mybir.dt.bfloat16
MULT = mybir.AluOpType.mult

LS = [0, 1, 2, 3, 0, 1, 2, 3]


# ----------------------------------------------------------------------------
# Host-side table preprocessing
# ----------------------------------------------------------------------------

def _build_tables(idx1, idx2, out_idx, cb_vals):
    """Build the factorized layout.

    Returns (iperm, e12, w, rows_map):
      iperm: (32,) permutation of in1 columns (iset-major).
      e12: (64, 6*128) bf16 -- E1a at cols a*128.. (a=0..3), E2b at
           cols (4+b)*128.. (b=0,1); rows index in12T partitions.
      w:   (128, 8*128) bf16 -- w[p, q*128+m] = coefficient for pair p
           (p = i_local*16 + j_local) into scratch out row q*128+m.
      rows_map: (1024,) int -- scratch row -> real out column.
    """
    idx1 = np.asarray(idx1, np.int64)
    idx2 = np.asarray(idx2, np.int64)
    out_idx = np.asarray(out_idx, np.int64)
    cb = np.asarray(cb_vals, np.float64)

    offs, blocks = 0, []
    for l in LS:
        blocks.append(list(range(offs, offs + 2 * l + 1)))
        offs += 2 * l + 1
    isets = [blocks[0] + blocks[3], blocks[1] + blocks[2],
             blocks[4] + blocks[7], blocks[5] + blocks[6]]
    jsets = [list(range(16)), list(range(16, 32))]
    imap = {c: (a, il) for a, s in enumerate(isets) for il, c in enumerate(s)}
    jmap = {c: (b, jl) for b, s in enumerate(jsets) for jl, c in enumerate(s)}

    out_q = {}
    for k in range(len(cb)):
        a, _ = imap[int(idx1[k])]
        b, _ = jmap[int(idx2[k])]
        q = a * 2 + b
        o = int(out_idx[k])
        assert out_q.setdefault(o, q) == q, "coupling crosses pair-tiles"
    rows_map = np.zeros(NQ * 128, np.int64)
    out_local = {}
    for q in range(NQ):
        outs = sorted(o for o, qq in out_q.items() if qq == q)
        assert len(outs) == 128, (q, len(outs))
        for m, o in enumerate(outs):
            out_local[o] = m
            rows_map[q * 128 + m] = o

    e12 = np.zeros((64, 6 * 128), np.float32)
    for a in range(4):
        for p in range(128):
            e12[a * 8 + p // 16, a * 128 + p] = 1.0
    for b in range(2):
        for p in range(128):
            e12[32 + b * 16 + p % 16, (4 + b) * 128 + p] = 1.0

    w = np.zeros((128, NQ * 128), np.float64)
    for k in range(len(cb)):
        a, il = imap[int(idx1[k])]
        b, jl = jmap[int(idx2[k])]
        q = a * 2 + b
        p = il * 16 + jl
        m = out_local[int(out_idx[k])]
        w[p, q * 128 + m] += cb[k]

    iperm = np.concatenate([np.asarray(s) for s in isets])
    bf = ml_dtypes.bfloat16
    return iperm, e12.astype(bf), w.astype(np.float32).astype(bf), rows_map


# ----------------------------------------------------------------------------
# Device program
# ----------------------------------------------------------------------------

def _build_bass():
    nc = bacc.Bacc("TRN2", target_bir_lowering=False)

    in12h = nc.dram_tensor("in12h", [64, BC], BF16, kind="ExternalInput")
    e12 = nc.dram_tensor("e12", [64, 6 * 128], BF16, kind="ExternalInput")
    wgt = nc.dram_tensor("wgt", [128, NQ * 128], BF16, kind="ExternalInput")
    outT = nc.dram_tensor("outT", [NQ * 128, BC], BF16, kind="ExternalOutput")

    # R slots 0-3 = R1 (isets), 4-5 = R2 (jsets).
    # Emission program: 16 R-matmuls up front (PE p-state ramp), then mains
    # interleaved with the last two R slots so the PE never drains. GPSIMD
    # (slow, SBUF-only) gets the two muls consumed last, emitted early.
    # 'R' = slot matmuls+casts, 'M' = DVE mul, 'MG' = GPSIMD mul,
    # 'Q' = main matmuls + out casts + DMA for pair-tile (a, b).
    program = [
        ('R', 0), ('R', 4), ('R', 1), ('MG', (1, 0)), ('R', 5), ('MG', (0, 1)),
        ('M', (0, 0)), ('M', (1, 1)),
        ('Q', (0, 0)), ('R', 2), ('M', (2, 0)), ('M', (2, 1)),
        ('Q', (1, 1)), ('R', 3), ('M', (3, 1)), ('M', (3, 0)),
        ('Q', (2, 0)), ('Q', (3, 1)), ('Q', (2, 1)), ('Q', (3, 0)),
        ('Q', (1, 0)), ('Q', (0, 1)),
    ]
    gps_muls = {(0, 1), (1, 0)}
    # cast engine per op: R slots mostly alternate ACT/DVE; slot 2 all ACT
    # (keeps DVE free for muls); out casts ~11 ACT / 5 DVE, tail parallel.
    r_cast_plan = {0: 'av', 4: 'av', 1: 'av', 5: 'av', 2: 'aa', 3: 'av'}
    o_cast_seq = 'aavaavaavaavaaav'

    with tile.TileContext(nc) as tc:
        with (
            tc.tile_pool(name="const", bufs=1) as const_pool,
            tc.tile_pool(name="work", bufs=1) as work_pool,
            tc.tile_pool(name="ps_r", bufs=2, space="PSUM") as ps_r_pool,
            tc.tile_pool(name="ps_o", bufs=2, space="PSUM") as ps_o_pool,
        ):
            # issue the input DMAs from four different engine queues so they
            # start in parallel instead of serializing on the Sync sequencer
            x_sb = work_pool.tile([64, BC], BF16)
            nc.sync.dma_start(out=x_sb[:, : BC // 2], in_=in12h.ap()[:, : BC // 2])
            nc.gpsimd.dma_start(out=x_sb[:, BC // 2 :], in_=in12h.ap()[:, BC // 2 :])
            e_sb = const_pool.tile([64, 6 * 128], BF16)
            nc.scalar.dma_start(out=e_sb[:], in_=e12.ap())
            w_sb = const_pool.tile([128, NQ * 128], BF16)
            nc.sync.dma_start(out=w_sb[:], in_=wgt.ap())



            r_sb = work_pool.tile([128, 6 * BC], BF16)
            u_sb = work_pool.tile([128, NQ * BC], BF16)
            osb = work_pool.tile([128, NQ * BC], BF16)

            def emit_cast(code, dst, ps):
                if code == 'a':
                    nc.scalar.copy(out=dst, in_=ps[:])
                else:
                    nc.vector.tensor_copy(dst, ps[:])

            oci = [0]
            for kind, arg in program:
                if kind == 'R':
                    slot = arg
                    for h in range(2):  # halves of BC: 2 chunks each
                        ps = ps_r_pool.tile([128, 2 * CHUNK], F32)
                        for ci in range(2):
                            c = h * 2 + ci
                            nc.tensor.matmul(
                                ps[:, ci * CHUNK : (ci + 1) * CHUNK],
                                lhsT=e_sb[:, slot * 128 : (slot + 1) * 128],
                                rhs=x_sb[:, c * CHUNK : (c + 1) * CHUNK],
                                start=True,
                                stop=True,
                            )
                        emit_cast(
                            r_cast_plan[slot][h],
                            r_sb[:, slot * BC + h * 1024 : slot * BC + (h + 1) * 1024],
                            ps,
                        )
                elif kind in ('M', 'MG'):
                    a, b = arg
                    q = a * 2 + b
                    out_ap = u_sb[:, q * BC : (q + 1) * BC]
                    in0 = r_sb[:, a * BC : (a + 1) * BC]
                    in1 = r_sb[:, (4 + b) * BC : (5 + b) * BC]
                    if kind == 'MG':
                        # Pool engine: only plain TensorTensor is supported
                        nc.gpsimd.tensor_mul(out_ap, in0, in1)
                    else:
                        nc.vector.tensor_mul(out_ap, in0, in1)
                else:  # 'Q': mains + out casts + per-half DMA
                    a, b = arg
                    q = a * 2 + b
                    for h in range(2):
                        ps = ps_o_pool.tile([128, 2 * CHUNK], F32)
                        for ci in range(2):
                            c = h * 2 + ci
                            nc.tensor.matmul(
                                ps[:, ci * CHUNK : (ci + 1) * CHUNK],
                                lhsT=w_sb[:, q * 128 : (q + 1) * 128],
                                rhs=u_sb[:, q * BC + c * CHUNK : q * BC + (c + 1) * CHUNK],
                                start=True,
                                stop=True,
                            )
                        emit_cast(
                            o_cast_seq[oci[0] % len(o_cast_seq)],
                            osb[:, q * BC + h * 1024 : q * BC + (h + 1) * 1024],
                            ps,
                        )
                        oci[0] += 1
                        nc.sync.dma_start(
                            out=outT.ap()[
                                q * 128 : (q + 1) * 128, h * 1024 : (h + 1) * 1024
                            ],
                            in_=osb[:, q * BC + h * 1024 : q * BC + (h + 1) * 1024],
                        )
    nc.compile()
    return nc


# ----------------------------------------------------------------------------
# Entry point
# ----------------------------------------------------------------------------

_CACHE = {}


def kernel(in1, in2, cb_vals, idx1, idx2, out_idx):
    in1 = np.ascontiguousarray(np.asarray(in1, np.float32))
    in2 = np.ascontiguousarray(np.asarray(in2, np.float32))

    key = (
        np.asarray(idx1).tobytes(),
        np.asarray(idx2).tobytes(),
        np.asarray(out_idx).tobytes(),
        np.asarray(cb_vals).tobytes(),
    )
    kh = hash(key)
    if kh not in _CACHE:
        iperm, e12, w, rows_map = _build_tables(idx1, idx2, out_idx, cb_vals)
        nc = _build_bass()
        _CACHE[kh] = (nc, iperm, e12, w, rows_map)
    nc, iperm, e12, w, rows_map = _CACHE[kh]

    bf = ml_dtypes.bfloat16
    in1p = in1[:, iperm]
    in_maps = []
    for core in range(N_CORES):
        sl = slice(core * BC, (core + 1) * BC)
        in12h = np.ascontiguousarray(
            np.concatenate([in1p[sl], in2[sl]], axis=1).T.astype(bf)
        )  # (64, BC)
        in_maps.append({"in12h": in12h, "e12": e12, "wgt": w})

    trace = bool(int(os.environ.get("KERNEL_TRACE", "0")))
    res = run_bass_kernel_spmd(
        nc, in_maps, core_ids=list(range(N_CORES)), trace=trace
    )
    kernel.last_results = res

    out = np.empty((B, DOUT), np.float32)
    for core in range(N_CORES):
        shard = res.results[core]["outT"]  # (1024, BC) bf16 scratch layout
        out[core * BC : (core + 1) * BC][:, rows_map] = (
            np.asarray(shard).astype(np.float32).T
        )
    return out
